# revision 54
# baseline (speedup 1.0000x reference)
"""Distributed Trainium2 kernel for nn_AttentionBsl (LN -> QKV -> 16-head
attention -> output projection) on 8 NeuronCores.  ~334-341us vs the
543us v1 baseline (rel err 4.1e-3 vs the 2e-2 gate).  The output
projection runs fully bf16 (wo cast on the idle Act engine under
phase B) to halve its LDWEIGHTS cost.

Sharding: token-parallel. Core j handles batch j//4, tokens
[512*(j%4), 512*(j%4+1)).  Each core layernorms its token slice, computes
K/V/Q projections for its tokens, AllGathers K and V (bf16), runs
attention for its 512 queries against all 2048 keys, and applies the
output projection.  Output shards are disjoint -> host concat.

Design notes (what made it fast):
- LN stats via bf16 ones-matmuls (f32 matmuls run at 1/4 PE rate; f32r
  rank-1s fail walrus codegen).  The affine is two f32 broadcast rank-1
  matmuls plus per-partition gamma/beta applied on DVE, overlapping the
  K projection per-ci.
- Weights stream through SBUF in 4KB/partition chunks with ci-outer
  projection loops holding one PSUM bank per output tile, so projections
  start as soon as the first chunk lands.
- The K/V AllGather (bf16, V carries a 65th ones column per head for
  softmax denominators) is split into four chunks fired in consumption
  order: K heads 0-7, V heads 0-7, K heads 8-15, V heads 8-15.  All cc
  staging/loads are row-contiguous 2D DMA patterns (multi-dim patterns
  cost 4-12us of descriptor issue).  tc.tile_wait_until stamps give the
  tile scheduler realistic collective-completion times; without them the
  static per-engine streams head-of-line block on gathers.
- attn@V: stationary = V-tile [128keys, 65], moving = all 512 queries
  (half the LDWEIGHTS of the at-stationary form, no PE transposes, and
  denominators land on PSUM partition 64).  attn@V of pair p interleaves
  with scores/exp of pair p+1 (one-pair lag) so the exp stream on the
  Act engine - the 109us/core lower bound that paces phase B - never
  directly precedes PE work of the same pair, keeping the PE pstate at
  2.4GHz.
- Softmax normalization: reciprocal_approx_fast of the denominator row
  (the accurate reciprocal costs 3.4us per call and sat on the critical
  path), broadcast via a 64-row ones matmul, applied on DVE.  Head B of
  each pair reaches its ao partitions via a partition-shifting SBUF DMA
  (DVE ops cannot cross partitions).
- A 16-byte dummy AllGather fires at t=0: the first CC op pays ~15us of
  warm-up, which the dummy absorbs behind the startup barrier (K1 then
  runs in 28us instead of 47).
- exp must stay on the Act engine: DVE/GpSimd have no exp, and any
  fp8/Schraudolph shortcut puts >2.9% rms noise on the attention
  weights, which transfers 1:1 to the output (softmax averaging does
  not reduce relative error).  bf16 everywhere on the attention path.
"""

import sys

if "/opt/trn_rl_repo" not in sys.path:
    sys.path.insert(0, "/opt/trn_rl_repo")

import numpy as np

DIM = 1024
SEQ = 2048
BATCH = 2
HEADS = 16
DH = 64
NCORES = 8
GROUP = 4          # cores per batch group
NT = SEQ // GROUP  # 512 tokens per core
P = 128
CT = DIM // P      # 8 contraction tiles
NPAIR = HEADS // 2  # 8 head pairs
KT = SEQ // P      # 16 key tiles
EPS = 1e-5
LAG = 2            # attnV trails scores by LAG k-tiles

_CACHE = {}


def _build(debug=False):
    import concourse.bass as bass  # noqa: F401
    import concourse.mybir as mybir
    import concourse.tile as tile
    from concourse import bacc

    f32 = mybir.dt.float32
    f32r = mybir.dt.float32r
    bf16 = mybir.dt.bfloat16
    AF = mybir.ActivationFunctionType
    ALU = mybir.AluOpType
    RG = [[0, 1, 2, 3], [4, 5, 6, 7]]

    nc = bacc.Bacc("TRN2", target_bir_lowering=False, debug=False,
                   num_devices=NCORES)

    x_in = nc.dram_tensor("x", [DIM, NT], f32, kind="ExternalInput")
    wq_in = nc.dram_tensor("wq", [DIM, DIM], f32r, kind="ExternalInput")
    wk_in = nc.dram_tensor("wk", [DIM, DIM], f32r, kind="ExternalInput")
    wv_in = nc.dram_tensor("wv", [DIM, DIM], f32r, kind="ExternalInput")
    wo_in = nc.dram_tensor("wo", [DIM, DIM], f32r, kind="ExternalInput")
    g_in = nc.dram_tensor("gamma", [1, DIM], f32, kind="ExternalInput")
    b_in = nc.dram_tensor("beta", [1, DIM], f32, kind="ExternalInput")
    out_ext = nc.dram_tensor("out", [DIM, NT], f32, kind="ExternalOutput")
    if debug:
        dbg_h = nc.dram_tensor("dbg_h", [DIM, NT], f32, kind="ExternalOutput")
        dbg_q = nc.dram_tensor("dbg_q", [DIM, NT], f32, kind="ExternalOutput")
        dbg_k = nc.dram_tensor("dbg_k", [P, SEQ], f32, kind="ExternalOutput")
        dbg_v = nc.dram_tensor("dbg_v", [P, HEADS * (DH + 1)], f32,
                               kind="ExternalOutput")
        dbg_ao = nc.dram_tensor("dbg_ao", [DIM, NT], f32, kind="ExternalOutput")

    # Collective bounce buffers.  K stored [o_row, token] bf16, viewed as
    # f32 pairs along token; V stored [token, (head, d+1)] bf16 with the
    # ones column included.  Six chunks, sized so the front of the chain
    # (what phase B needs first) lands earliest:
    #   K1a (pairs 0-1) / K1b (pairs 2-3) / V q0 (heads 0-3) /
    #   V q1 (heads 4-7) / K2 (pairs 4-7) / V2 (heads 8-15).
    cc_dummy_in = nc.dram_tensor("ccdin", [4, 1], f32)
    cc_dummy_out = nc.dram_tensor("ccdout", [16, 1], f32)
    cck_in = [nc.dram_tensor("cck1i", [4 * P, NT // 2], f32),
              nc.dram_tensor("cck2i", [4 * P, NT // 2], f32)]
    cck_out = [nc.dram_tensor("cck1o", [GROUP * 4 * P, NT // 2], f32),
               nc.dram_tensor("cck2o", [GROUP * 4 * P, NT // 2], f32)]
    ccv_in = [nc.dram_tensor("ccv1i", [4 * P, 260], f32),
              nc.dram_tensor("ccv2i", [4 * P, 260], f32)]
    ccv_out = [nc.dram_tensor("ccv1o", [GROUP * 4 * P, 260], f32),
               nc.dram_tensor("ccv2o", [GROUP * 4 * P, 260], f32)]

    with tile.TileContext(nc) as tc:
        with (
            tc.tile_pool(name="const", bufs=1) as constp,
            tc.tile_pool(name="qp", bufs=NPAIR) as qpool,
            tc.tile_pool(name="kgp", bufs=NPAIR) as kgp,
            tc.tile_pool(name="vgp", bufs=KT) as vgp,
            tc.tile_pool(name="aop", bufs=CT) as aopool,
        ):
            # Warm the collective path: the first CC op pays ~15us extra;
            # let a 16-byte dummy absorb it behind the startup barrier.
            nc.gpsimd.collective_compute(
                "AllGather", ALU.bypass, replica_groups=RG,
                ins=[cc_dummy_in.ap().opt()], outs=[cc_dummy_out.ap().opt()])

            # ---- constants ----
            # gamma/beta as per-partition columns: gcol[p, ci] = gamma[ci*P+p]
            gcol = constp.tile([P, CT], f32)
            nc.sync.dma_start(
                out=gcol[:],
                in_=g_in.ap().rearrange("o (c p) -> p c o", p=P)[:, :, 0])
            bcol = constp.tile([P, CT], f32)
            nc.sync.dma_start(
                out=bcol[:],
                in_=b_in.ap().rearrange("o (c p) -> p c o", p=P)[:, :, 0])
            ones_col = constp.tile([P, 1], bf16)
            nc.gpsimd.memset(ones_col[:], 1.0)
            ones_rowp = constp.tile([1, P], f32)
            nc.gpsimd.memset(ones_rowp[:], 1.0)
            ones_row64 = constp.tile([1, DH], bf16)
            nc.gpsimd.memset(ones_row64[:], 1.0)


            # =========================================================
            # Phase A: LayerNorm -> K/V projections (-> AllGather) -> Q.
            # =========================================================
            with (
                tc.tile_pool(name="xh", bufs=1) as xhp,
                tc.tile_pool(name="wp", bufs=3) as wpool,
                tc.tile_pool(name="stage", bufs=1) as stagep,
            ):
                x_t = []
                for ci in range(CT):
                    t = xhp.tile([P, NT], f32, tag="x", name="x_t", bufs=CT)
                    nc.sync.dma_start(out=t[:],
                                      in_=x_in[ci * P:(ci + 1) * P, :])
                    x_t.append(t)
                h_t = [xhp.tile([P, NT], f32r, tag="h", name="h_t", bufs=CT)
                       for _ in range(CT)]

                with (
                    tc.tile_pool(name="sqp", bufs=3) as sqp,
                    tc.tile_pool(name="stats", bufs=1) as statp,
                    tc.tile_pool(name="tup", bufs=3) as tup,
                    tc.tile_pool(name="lnps", bufs=2, space="PSUM") as lnps,
                    tc.tile_pool(name="abps", bufs=1, space="PSUM") as abps,
                ):
                    ps_sum = lnps.tile([1, NT], f32)
                    ps_sq = lnps.tile([1, NT], f32)
                    for ci in range(CT):
                        xb = sqp.tile([P, NT], bf16, tag="xb", name="xb_t")
                        nc.vector.tensor_copy(xb[:], x_t[ci][:])
                        sq = sqp.tile([P, NT], bf16, tag="sq", name="sq_t")
                        nc.scalar.activation(sq[:], x_t[ci][:], AF.Square)
                        nc.tensor.matmul(ps_sum[:], ones_col[:], xb[:],
                                         start=(ci == 0), stop=(ci == CT - 1))
                        nc.tensor.matmul(ps_sq[:], ones_col[:], sq[:],
                                         start=(ci == 0), stop=(ci == CT - 1))

                    mean = statp.tile([1, NT], f32, tag="st", name="mean", bufs=6)
                    nc.vector.tensor_scalar_mul(mean[:], ps_sum[:], 1.0 / DIM)
                    var = statp.tile([1, NT], f32, tag="st", name="var", bufs=6)
                    nc.vector.tensor_scalar_mul(var[:], ps_sq[:], 1.0 / DIM)
                    m2 = statp.tile([1, NT], f32, tag="st", name="m2", bufs=6)
                    nc.vector.tensor_tensor(m2[:], mean[:], mean[:], ALU.mult)
                    nc.vector.tensor_tensor(var[:], var[:], m2[:],
                                            ALU.subtract)
                    nc.vector.tensor_scalar_add(var[:], var[:], EPS)
                    rv = statp.tile([1, NT], f32, tag="st", name="rv", bufs=6)
                    nc.vector.reciprocal_approx_fast(rv[:], var[:])
                    rstd = statp.tile([1, NT], f32, tag="st", name="rstd", bufs=6)
                    nc.scalar.activation(rstd[:], rv[:], AF.Sqrt)
                    nb = statp.tile([1, NT], f32, tag="st", name="nb", bufs=6)
                    nc.vector.tensor_tensor(nb[:], mean[:], rstd[:], ALU.mult)
                    nc.vector.tensor_scalar_mul(nb[:], nb[:], -1.0)

                    # Broadcast rows: bc_r = 1 x rstd, bc_m = 1 x (-mean*rstd)
                    ab = abps.tile([P, 2 * NT], f32, tag="ab", name="ab")
                    nc.tensor.matmul(ab[:, 0:NT], ones_rowp[:], rstd[:],
                                     start=True, stop=True)
                    nc.tensor.matmul(ab[:, NT:2 * NT], ones_rowp[:], nb[:],
                                     start=True, stop=True)
                    # h = gamma*(x*bc_r) + (gamma*bc_m + beta)
                    for ci in range(CT):
                        gsl = gcol[:, ci:ci + 1]
                        bsl = bcol[:, ci:ci + 1]
                        tt = tup.tile([P, NT], f32, tag="tu", name="t_t")
                        nc.vector.scalar_tensor_tensor(
                            tt[:], x_t[ci][:], gsl, ab[:, 0:NT],
                            ALU.mult, ALU.mult)
                        ut = tup.tile([P, NT], f32, tag="tu", name="u_t")
                        nc.vector.tensor_scalar(
                            ut[:], ab[:, NT:2 * NT], gsl, bsl,
                            ALU.mult, ALU.add)
                        nc.vector.tensor_tensor(h_t[ci][:], tt[:], ut[:],
                                                ALU.add)

                # ---- K projection in two ot-pass halves so the first K
                # AllGather fires as early as possible ----
                kstage = [stagep.tile([P, CT // 2, NT], bf16, tag="kst",
                                      name=f"kstage{i}", bufs=2)
                          for i in range(2)]
                with tc.tile_pool(name="kjps", bufs=CT // 2,
                                  space="PSUM") as kjps:
                    for half in range(2):
                        kps4 = [kjps.tile([P, NT], f32, tag="pj", name="kps",
                                          bufs=4)
                                for _ in range(CT // 2)]
                        kps = [None] * (half * 4) + kps4 \
                            if half else kps4 + [None] * 4
                        cols = slice(half * NT, (half + 1) * NT)
                        for ci in range(CT):
                            wc = wpool.tile([P, NT], f32r, tag="w",
                                            name="wk_c", bufs=6)
                            nc.sync.dma_start(
                                out=wc[:],
                                in_=wk_in[ci * P:(ci + 1) * P, cols])
                            for o4 in range(CT // 2):
                                ot = half * 4 + o4
                                nc.tensor.matmul(
                                    kps[ot][:], wc[:, o4 * P:(o4 + 1) * P],
                                    h_t[ci][:],
                                    start=(ci == 0), stop=(ci == CT - 1))
                        for o4 in range(CT // 2):
                            ot = half * 4 + o4
                            nc.scalar.activation(kstage[half][:, o4, :],
                                                 kps[ot][:], AF.Copy)
                        if half == 0:
                            with tc.tile_wait_until(0.042):
                                nc.sync.dma_start(
                                    out=cck_in[0].ap().bitcast(
                                        bf16).rearrange(
                                        "(p c) n -> p (c n)", p=P),
                                    in_=kstage[0][:].rearrange(
                                        "p c n -> p (c n)"))
                                # Fire K heads 0-7 immediately.
                                nc.gpsimd.collective_compute(
                                    "AllGather", ALU.bypass,
                                    replica_groups=RG,
                                    ins=[cck_in[0].ap().opt()],
                                    outs=[cck_out[0].ap().opt()])

                # ---- V projection (h-stationary, ci-outer) ----
                with tc.tile_pool(name="vjps", bufs=GROUP,
                                  space="PSUM") as vjps:
                    vps = [vjps.tile([P, DIM], f32, tag="vpj", name="vps")
                           for _ in range(GROUP)]
                    for ci in range(CT):
                        wc = wpool.tile([P, DIM], f32r, tag="wv", name="wv_c")
                        nc.sync.dma_start(out=wc[:],
                                          in_=wv_in[ci * P:(ci + 1) * P, :])
                        for tt in range(GROUP):
                            for hf in range(2):
                                nc.tensor.matmul(
                                    vps[tt][:, hf * NT:(hf + 1) * NT],
                                    h_t[ci][:, tt * P:(tt + 1) * P],
                                    wc[:, hf * NT:(hf + 1) * NT],
                                    start=(ci == 0), stop=(ci == CT - 1))
                    vstg = [stagep.tile([P, GROUP, 8, DH + 1], bf16,
                                        tag="vst", name=f"vstg{i}", bufs=2)
                            for i in range(2)]
                    for half in range(2):
                        nc.gpsimd.memset(vstg[half][:, :, :, DH:DH + 1], 1.0)
                        for tt in range(GROUP):
                            nc.vector.tensor_copy(
                                vstg[half][:, tt, :, 0:DH],
                                vps[tt][:, half * NT:(half + 1) * NT]
                                .rearrange("p (h d) -> p h d", h=8))
                    with tc.tile_wait_until(0.070):
                        nc.sync.dma_start(
                            out=ccv_in[0].ap().bitcast(bf16).rearrange(
                                "(p t) w -> p (t w)", p=P),
                            in_=vstg[0][:].rearrange("p t h d -> p (t h d)"))
                    # V heads 0-7: second in the chain (pairs 0-3 attnV).
                    with tc.tile_wait_until(0.065):
                        nc.gpsimd.collective_compute(
                            "AllGather", ALU.bypass, replica_groups=RG,
                            ins=[ccv_in[0].ap().opt()],
                            outs=[ccv_out[0].ap().opt()])

                # ---- Q projection (ci-outer) ----
                with tc.tile_pool(name="qjps", bufs=CT, space="PSUM") as qjps:
                    qps = [qjps.tile([P, NT], f32, tag="qj", name="qps")
                           for _ in range(CT)]
                    for ci in range(CT):
                        wc = wpool.tile([P, DIM], f32r, tag="wq", name="wq_c")
                        with tc.tile_wait_until(0.050 + 0.002 * ci):
                            nc.sync.dma_start(
                                out=wc[:],
                                in_=wq_in[ci * P:(ci + 1) * P, :])
                        for ot in range(CT):
                            nc.tensor.matmul(
                                qps[ot][:], wc[:, ot * P:(ot + 1) * P],
                                h_t[ci][:],
                                start=(ci == 0), stop=(ci == CT - 1))
                    q_t = []
                    for ot in range(CT):
                        qt_ = qpool.tile([P, NT], bf16, tag="q", name="q_t")
                        nc.scalar.activation(qt_[:], qps[ot][:], AF.Copy)
                        q_t.append(qt_)

                # K heads 8-15 and V heads 8-15 staged late so the
                # scheduler keeps them behind the front of the chain.
                with tc.tile_wait_until(0.054):
                    nc.sync.dma_start(
                        out=cck_in[1].ap().bitcast(bf16).rearrange(
                            "(p c) n -> p (c n)", p=P),
                        in_=kstage[1][:].rearrange("p c n -> p (c n)"))
                with tc.tile_wait_until(0.092):
                    nc.gpsimd.collective_compute(
                        "AllGather", ALU.bypass, replica_groups=RG,
                        ins=[cck_in[1].ap().opt()],
                        outs=[cck_out[1].ap().opt()])
                with tc.tile_wait_until(0.072):
                    nc.sync.dma_start(
                        out=ccv_in[1].ap().bitcast(bf16).rearrange(
                            "(p t) w -> p (t w)", p=P),
                        in_=vstg[1][:].rearrange("p t h d -> p (t h d)"))
                with tc.tile_wait_until(0.125):
                    nc.gpsimd.collective_compute(
                        "AllGather", ALU.bypass, replica_groups=RG,
                        ins=[ccv_in[1].ap().opt()],
                        outs=[ccv_out[1].ap().opt()])

            # ---- gathered K/V into SBUF, in the order phase B needs.
            # All loads are plain 2D row-contiguous patterns (cheap SP
            # descriptor issue).
            k_g = [[None] * GROUP for _ in range(NPAIR)]
            v_g = [[None] * KT for _ in range(2)]

            def load_kg(hp):
                for r in range(GROUP):
                    t = kgp.tile([P, NT], bf16, tag="kg", name="k_g",
                                 bufs=NPAIR * GROUP)
                    src = cck_out[hp // 4][r * 4 * P:(r + 1) * 4 * P, :]
                    src = src.bitcast(bf16).rearrange("(p c) n -> p c n",
                                                      p=P)
                    nc.sync.dma_start(out=t[:], in_=src[:, hp % 4, :])
                    k_g[hp][r] = t

            def load_vg(half):
                for kt in range(KT):
                    r, tt = divmod(kt, GROUP)
                    t = vgp.tile([P, 8, DH + 1], bf16, tag=f"vg{half}",
                                 name="v_g", bufs=KT)
                    src = ccv_out[half][r * 4 * P:(r + 1) * 4 * P, :]
                    src = src.bitcast(bf16).rearrange("(p t) w -> p t w",
                                                      p=P)
                    nc.sync.dma_start(
                        out=t[:].rearrange("p h d -> p (h d)"),
                        in_=src[:, tt, :])
                    v_g[half][kt] = t

            # Stamp the loads with realistic collective-completion times so
            # the static schedule interleaves attnV where data will exist.
            with tc.tile_wait_until(0.092):
                for hp in range(4):
                    load_kg(hp)
            with tc.tile_wait_until(0.128):
                load_vg(0)
            with tc.tile_wait_until(0.162):
                for hp in range(4, NPAIR):
                    load_kg(hp)
            with tc.tile_wait_until(0.196):
                load_vg(1)


            # =========================================================
            # Phase B: attention.
            #   scT[k, q] = K^T q  (per pair, two 64-contraction matmuls)
            #   at = exp(scT / 8)          (ScalarE, bf16)
            #   av[d(+den), q] += V_aug^T @ at   (v-stationary matmuls)
            #   ao[hd, q] = av[d, q] * recip(den)[q]
            # =========================================================
            ao_t = [aopool.tile([P, NT], bf16, tag="ao", name="ao")
                    for _ in range(CT)]
            with (
                tc.tile_pool(name="atp", bufs=8) as atp,
                tc.tile_pool(name="rcp", bufs=1) as rcp,
                tc.tile_pool(name="tmpb", bufs=2) as tmpbp,
                tc.tile_pool(name="scps", bufs=2, space="PSUM") as scps,
                tc.tile_pool(name="avps", bufs=3, space="PSUM") as avps,
                tc.tile_pool(name="bcps", bufs=1, space="PSUM") as bcps,
            ):
                prev = None  # (hp, av_pair) awaiting normalize

                def normalize(hp, av_pair):
                    for hi in range(2):
                        den = rcp.tile([1, NT], f32, tag="den",
                                       name="den", bufs=4)
                        nc.vector.tensor_copy(den[:],
                                              av_pair[hi][DH:DH + 1, :])
                        rc = rcp.tile([1, NT], f32, tag="rc", name="rc",
                                      bufs=4)
                        nc.vector.reciprocal_approx_fast(rc[:], den[:])
                        rcb = rcp.tile([1, NT], bf16, tag="rcb", name="rcb",
                                       bufs=4)
                        nc.vector.tensor_copy(rcb[:], rc[:])
                        bc = bcps.tile([DH, NT], f32, tag="bc", name="bc")
                        nc.tensor.matmul(bc[:], ones_row64[:], rcb[:],
                                         start=True, stop=True)
                        bcs = tmpbp.tile([DH, NT], f32, tag="bcs",
                                         name="bcs", bufs=2)
                        nc.vector.tensor_copy(bcs[:], bc[:])
                        if hi == 0:
                            nc.vector.tensor_tensor(
                                ao_t[hp][0:DH, :], av_pair[0][0:DH, :],
                                bcs[:], ALU.mult)
                        else:
                            tmpb = tmpbp.tile([DH, NT], bf16, tag="tb",
                                              name="tmpb")
                            nc.vector.tensor_tensor(
                                tmpb[:], av_pair[1][0:DH, :], bcs[:],
                                ALU.mult)
                            nc.sync.dma_start(out=ao_t[hp][DH:P, :],
                                              in_=tmpb[:])

                def attnv(hp, kt, av, at_tile):
                    for hi in range(2):
                        hg = hp * 2 + hi
                        vg = v_g[hg // 8][kt]
                        nc.tensor.matmul(
                            av[hi][:],
                            vg[:, hg % 8, :],
                            at_tile[:, hi * NT:(hi + 1) * NT],
                            start=(kt == 0), stop=(kt == KT - 1))

                # attnV of pair p runs interleaved with the scores/exp of
                # pair p+1 (one-pair lag, 2-kt offset), so the Act engine is
                # never the PE's direct predecessor within a pair, and the
                # previous pair's normalize lands between kt=1 and kt=2 —
                # off the PE queue head, after its reciprocal had time to
                # run.
                at_store = {}
                for hp in range(NPAIR + 1):
                    if hp > 0:
                        av = [avps.tile([DH + 1, NT], f32, tag="av",
                                        name="av")
                              for _ in range(2)]
                    for kt in range(KT):
                        if hp < NPAIR:
                            sc = scps.tile([P, 2 * NT], f32, tag="sc",
                                           name="sc")
                            r, tt = divmod(kt, GROUP)
                            ksl = k_g[hp][r][:, tt * P:(tt + 1) * P]
                            nc.tensor.matmul(
                                sc[:, 0:NT],
                                ksl[0:DH, :],
                                q_t[hp][0:DH, :], start=True, stop=True)
                            nc.tensor.matmul(
                                sc[:, NT:2 * NT],
                                ksl[DH:P, :],
                                q_t[hp][DH:P, :], start=True, stop=True)
                            at = atp.tile([P, 2 * NT], bf16, tag="at",
                                          name="at", bufs=20)
                            nc.scalar.activation(
                                at[:], sc[:], AF.Exp,
                                scale=float(1.0 / np.sqrt(DH)))
                            at_store[(hp, kt)] = at
                        if kt == 2 and prev is not None:
                            normalize(*prev)
                            prev = None
                        if hp > 0 and kt >= 2:
                            attnv(hp - 1, kt - 2, av,
                                  at_store.pop((hp - 1, kt - 2)))
                    if hp > 0:
                        for kt in range(KT - 2, KT):
                            attnv(hp - 1, kt, av,
                                  at_store.pop((hp - 1, kt)))
                        prev = (hp - 1, av)
                normalize(*prev)

            if debug:
                for ci in range(CT):
                    nc.sync.dma_start(out=dbg_ao[ci * P:(ci + 1) * P, :],
                                      in_=ao_t[ci][:])

            # =========================================================
            # Phase C: output projection (wo streamed).
            # =========================================================
            with (
                tc.tile_pool(name="wop", bufs=3) as wop,
                tc.tile_pool(name="outsb", bufs=2) as outp,
                tc.tile_pool(name="ops", bufs=CT, space="PSUM") as ops,
            ):
                ops_t = [ops.tile([P, NT], f32, tag="o", name="ops_t")
                         for _ in range(CT)]
                for ci in range(CT):
                    wc = wop.tile([P, DIM], f32r, tag="wo", name="wo_c")
                    with tc.tile_wait_until(0.22 + 0.004 * ci):
                        nc.sync.dma_start(out=wc[:],
                                          in_=wo_in[ci * P:(ci + 1) * P, :])
                    wb = wop.tile([P, DIM], bf16, tag="wob", name="wo_b",
                                  bufs=3)
                    with tc.tile_wait_until(0.24 + 0.004 * ci):
                        nc.scalar.activation(wb[:], wc[:].bitcast(f32),
                                             AF.Copy)
                    for ot in range(CT):
                        nc.tensor.matmul(
                            ops_t[ot][:], wb[:, ot * P:(ot + 1) * P],
                            ao_t[ci][:],
                            start=(ci == 0), stop=(ci == CT - 1))
                for ot in range(CT):
                    ost = outp.tile([P, NT], f32, tag="ou", name="ost")
                    nc.vector.tensor_copy(ost[:], ops_t[ot][:])
                    nc.sync.dma_start(out=out_ext[ot * P:(ot + 1) * P, :],
                                      in_=ost[:])

    nc.compile()
    return nc


def _get_nc(debug=False):
    key = ("nc", debug)
    if key not in _CACHE:
        _CACHE[key] = _build(debug)
    return _CACHE[key]


def kernel(x, w_qkv, w_out, ln_gamma, ln_beta, _profile=False, _debug=False):
    from concourse.bass_utils import run_bass_kernel_spmd

    x = np.asarray(x, np.float32)
    w_qkv = np.asarray(w_qkv, np.float32)
    w_out = np.asarray(w_out, np.float32)
    ln_gamma = np.asarray(ln_gamma, np.float32)
    ln_beta = np.asarray(ln_beta, np.float32)

    wq = np.ascontiguousarray(w_qkv[0:DIM].T)
    wk = np.ascontiguousarray(w_qkv[DIM:2 * DIM].T)
    wv = np.ascontiguousarray(w_qkv[2 * DIM:3 * DIM].T)
    wo = np.ascontiguousarray(w_out.T)
    grow = np.ascontiguousarray(ln_gamma.reshape(1, DIM))
    brow = np.ascontiguousarray(ln_beta.reshape(1, DIM))

    in_maps = []
    for j in range(NCORES):
        b, c = divmod(j, GROUP)
        in_maps.append({
            "x": np.ascontiguousarray(x[:, c * NT:(c + 1) * NT, b]),
            "wq": wq, "wk": wk, "wv": wv, "wo": wo,
            "gamma": grow, "beta": brow,
        })

    nc = _get_nc(_debug)
    res = run_bass_kernel_spmd(nc, in_maps, core_ids=list(range(NCORES)),
                               trace=_profile)
    if _profile:
        _CACHE["last_result"] = res

    out = np.empty((DIM, SEQ, BATCH), np.float32)
    for j in range(NCORES):
        b, c = divmod(j, GROUP)
        out[:, c * NT:(c + 1) * NT, b] = res.results[j]["out"]
    if _debug:
        _CACHE["dbg"] = res.results
    return out
